# revision 32
# baseline (speedup 1.0000x reference)
"""Trainium2 Bass kernel for MultiHeadCrossAttention.

Problem: y = proj(softmax(mask(q @ k^T / sqrt(Dh))) @ v) with
  x: (16, 1024, 1024) f32, cond: (16, 120, 1024) f32, mask: (16, 120) i32,
  Wq: (1024, 1024), Wkv: (2048, 1024), Wp: (1024, 1024); H=16 heads, Dh=64.
  Biases are all zeros per the problem spec and are skipped.

Sharding: pure data-parallel over batch B=16 -> 2 batches per core on 8
NeuronCores. No collectives; each core runs the same program (SPMD) on its
batch shard plus the full (replicated) weights.

Host-side prep (layout only): weights are staged to device DRAM already
transposed (W.T, f32) so the device loads W^T with natural, 512B-contiguous
strided DMA instead of on-chip XBAR transposition. Output is produced bf16
on device and upcast to f32 on host.

Per-core dataflow (everything "transposed" so each matmul contracts over the
partition dim):
  WqT/WkvT/WpT f32 block loads (SP ring) -> DVE cast to bf16 (resident)
  x f32 stage loads (ACT ring) -> DVE cast -> XBAR transpose (DVE ring)
  QT = WqT.T @ xT            [co, n]
  KT = WkvT(k).T @ condT     [co, l] (both batches merged, zero-padded to
                                      128-col blocks per batch)
  V  = condT.T @ WkvT(v)     [l, co]
  sT_h = KT_h.T @ QT_h       [l(128 rows, 8 zero-pad), n]
  e = Exp(sT/8 + maskbias)   (ACT; pad rows get -50 bias -> e ~ 0)
  R = partition_all_reduce(e01)  (GpSimd: softmax denominators broadcast to
                                  all 128 partitions -- off the PE)
  o~T_h = V_h.T @ e_h        [d, n] (head pairs via PE col-tiling)
  onormT = o~T * recip(R windows)  (DVE recip + DVE/GpSimd muls)
  y = onormT.T @ WpT         [n, co] -> bf16 SBUF -> DRAM.

Emission interleaves unit u's attention with unit u-1's out-projection and
unit u+1's Q-projection so the PE stream stays dense.
"""

import sys

for _p in ("/opt/trn_rl_repo", "/opt/pypackages"):
    if _p not in sys.path:
        sys.path.append(_p)

import numpy as np

B = 16
N_CORES = 8
B_PER_CORE = B // N_CORES  # 2
N = 1024
C = 1024
L = 120
H = 16
DH = C // H  # 64
SCALE = DH ** -0.5  # 0.125

KC = C // 128  # 8 c-chunks of 128
HP = H // 2  # 8 head pairs
NJ = 2  # n-halves per batch
NHALF = N // NJ  # 512
NEG = -50.0  # masked-logit bias; exp(s/8 - 50) ~ 0 vs reference's -inf

# Softmax row-sum strategy: True = GpSimd partition_all_reduce (PE saves the
# ones-broadcast matmuls), False = baseline-style PE ones-matmul broadcast.
USE_GPSIMD_SOFTMAX = False
DEBUG_DUMPS = False

_CACHE = {}


def _build_nc():
    import concourse.mybir as mybir
    import concourse.tile as tile
    from concourse import bacc, bass_isa

    FP = mybir.dt.float32
    BF = mybir.dt.bfloat16
    I32 = mybir.dt.int32
    Exp = mybir.ActivationFunctionType.Exp
    Alu = mybir.AluOpType

    nc = bacc.Bacc("TRN2", target_bir_lowering=False, debug=False)

    x_d = nc.dram_tensor("x", [B_PER_CORE, N, C], FP, kind="ExternalInput").ap()
    cond_d = nc.dram_tensor("cond", [B_PER_CORE, L, C], FP, kind="ExternalInput").ap()
    mask_d = nc.dram_tensor("mask", [B_PER_CORE, L], I32, kind="ExternalInput").ap()
    # Weights arrive HOST-TRANSPOSED: WqT = Wq.T [C, C], WkvT = Wkv.T
    # [C, 2C] (cols 0:C = Wk^T, C:2C = Wv^T), WpT = Wp.T [C, C].
    wqT_d = nc.dram_tensor("WqT", [C, C], FP, kind="ExternalInput").ap()
    wkvT_d = nc.dram_tensor("WkvT", [C, 2 * C], FP, kind="ExternalInput").ap()
    wpT_d = nc.dram_tensor("WpT", [C, C], FP, kind="ExternalInput").ap()
    out_d = nc.dram_tensor("out", [B_PER_CORE, N, C], BF, kind="ExternalOutput").ap()
    dbg = {}
    if DEBUG_DUMPS:
        for nm, shp, dt in [
            ("dbg_qT", [128, KC, NHALF], BF),
            ("dbg_ktT", [128, KC, 256], BF),
            ("dbg_vsb0", [128, C], BF),
            ("dbg_e01", [128, 1024], BF),
            ("dbg_R", [128, 1024], FP),
            ("dbg_onormT", [128, KC, NHALF], BF),
            ("dbg_xT", [128, KC, NHALF], BF),
        ]:
            dbg[nm] = nc.dram_tensor(nm, shp, dt, kind="ExternalOutput").ap()

    with tile.TileContext(nc) as tc:
        with (
            tc.tile_pool(name="wt", bufs=1) as wt,
            tc.tile_pool(name="wst", bufs=6) as wst,
            tc.tile_pool(name="xst", bufs=4) as xst,
            tc.tile_pool(name="act", bufs=2) as act,
            tc.tile_pool(name="sm", bufs=3) as sm,
            tc.tile_pool(name="ps", bufs=8, space="PSUM") as ps,
        ):
            # ---- resident bf16 transposed weights ----
            wqT = wt.tile([128, KC, C], BF, tag="wqT", name="wqT")
            wkvT = wt.tile([128, KC, 2 * C], BF, tag="wkvT", name="wkvT")
            wpT = wt.tile([128, KC, C], BF, tag="wpT", name="wpT")
            # per-core resident KV state (both batches)
            condT = wt.tile([128, KC, 256], BF, tag="condT", name="condT")
            ktT = wt.tile([128, KC, 256], BF, tag="ktT", name="ktT")
            vsbs = [
                wt.tile([128, C], BF, tag=f"vsb{b}", name=f"vsb{b}")
                for b in range(B_PER_CORE)
            ]
            mbs = [
                wt.tile([128, 1], FP, tag=f"mb{b}", name=f"mb{b}")
                for b in range(B_PER_CORE)
            ]
            if not USE_GPSIMD_SOFTMAX:
                ones_t = wt.tile([128, DH], BF, tag="ones_t", name="ones_t")
                nc.vector.memset(ones_t, 1.0)

            def w_load_chunk(dram2d, dstT, kc, half=None):
                # one kc row-chunk of a host-transposed weight: [128, 1024]
                # with 4KB-contiguous rows (full DMA descriptor rate), then
                # DVE cast into the resident bf16 tile. half selects the
                # K (0) / V (1) column half of WkvT.
                wf = wst.tile([128, C], FP, tag="wf", name="wf")
                src = dram2d[kc * 128 : (kc + 1) * 128, :]
                dst = dstT[:, kc, :]
                if half is not None:
                    src = src[:, half * C : (half + 1) * C]
                    dst = dstT[:, kc, half * C : (half + 1) * C]
                nc.sync.dma_start(out=wf[:], in_=src)
                if (kc + (0 if half is None else half)) % 2 == 0:
                    nc.vector.tensor_copy(out=dst, in_=wf[:])
                else:
                    nc.scalar.copy(out=dst, in_=wf[:])

            # ---- per-(batch, n-half) state ----
            units = [(b, j) for b in range(B_PER_CORE) for j in range(NJ)]
            xTs = {}
            qTs = {}

            def x_alloc(u):
                xTs[u] = act.tile([128, KC, NHALF], BF, tag="xT", name="xT")

            x_stage_bufs = {}

            def x_load(u):
                # whole n-half of x in ONE DMA (4KB rows): [128, 4, 1024]
                b, j = units[u]
                xf = xst.tile([128, 4, C], FP, tag="xfb", name="xfb", bufs=1)
                nc.scalar.dma_start(
                    out=xf[:],
                    in_=x_d[b, j * NHALF : (j + 1) * NHALF, :].rearrange(
                        "(s p) c -> p s c", p=128
                    ),
                )
                x_stage_bufs[u] = xf

            def x_cast(u, s):
                xf = x_stage_bufs[u]
                xb = xst.tile([128, C], BF, tag="xb", name="xb")
                nc.vector.tensor_copy(out=xb[:], in_=xf[:, s, :])
                x_stage_bufs[(u, s)] = xb

            def x_xpose(u, s, ring):
                xb = x_stage_bufs.pop((u, s))
                ring.dma_start_transpose(
                    xTs[u][:, :, s * 128 : (s + 1) * 128], xb[:]
                )

            def cond_load(b):
                # cond rows 0:120 (pad rows zeroed) -> bf16 -> XBAR into
                # condT cols [b*128, b*128+128)
                cf = xst.tile([128, C], FP, tag="cf", name="cond_f", bufs=1)
                nc.vector.memset(cf[:], 0.0)
                nc.sync.dma_start(out=cf[:L, :], in_=cond_d[b])
                cb = xst.tile([128, C], BF, tag="cb", name="cond_b", bufs=1)
                nc.vector.tensor_copy(out=cb[:], in_=cf[:])
                nc.scalar.dma_start_transpose(
                    condT[:, :, b * 128 : (b + 1) * 128], cb[:]
                )

            def mask_load(b):
                mi = wst.tile([128, 1], I32, tag="mi", name="mi")
                nc.sync.dma_start(out=mi[:L, :], in_=mask_d[b][:, None])
                mb = mbs[b]
                nc.vector.memset(mb[:], NEG)
                nc.vector.tensor_copy(out=mb[:L, :], in_=mi[:L, :])
                nc.vector.tensor_scalar(
                    mb[:L, :], mb[:L, :], -NEG, NEG, Alu.mult, Alu.add
                )

            def q_proj_chunk(u, m):
                # one 128-row chunk m of QT for unit u (8 accumulating MMs)
                if m == 0:
                    qTs[u] = act.tile([128, KC, NHALF], BF, tag="qT", name="qT")
                xT, qT = xTs[u], qTs[u]
                pt = ps.tile([128, 512], FP, tag="ps", name="q_ps")
                for kc in range(KC):
                    nc.tensor.matmul(
                        pt[:],
                        lhsT=wqT[:, kc, m * 128 : (m + 1) * 128],
                        rhs=xT[:, kc, :],
                        start=(kc == 0),
                        stop=(kc == KC - 1),
                    )
                nc.vector.tensor_copy(out=qT[:, m, :], in_=pt[:])

            def k_proj_chunk(m):
                # both batches at once: rhs N=248 (b0 cols 0:120 + 8 zero
                # pads, b1 cols 128:248); ktT pads pre-zeroed.
                pt = ps.tile([128, 512], FP, tag="ps", name="kt_ps")
                for kc in range(KC):
                    nc.tensor.matmul(
                        pt[:, 0:248],
                        lhsT=wkvT[:, kc, m * 128 : (m + 1) * 128],
                        rhs=condT[:, kc, 0:248],
                        start=(kc == 0),
                        stop=(kc == KC - 1),
                    )
                nc.scalar.copy(
                    out=ktT[:, m, :].rearrange("p (s k) -> p s k", s=2)[
                        :, :, 0:120
                    ],
                    in_=pt[:, 0:256].rearrange("p (s k) -> p s k", s=2)[
                        :, :, 0:120
                    ],
                )

            def v_proj_chunk(b, ch):
                pt = ps.tile([128, 512], FP, tag="ps", name="v_ps")
                for kc in range(KC):
                    nc.tensor.matmul(
                        pt[:L, :],
                        lhsT=condT[:, kc, b * 128 : b * 128 + L],
                        rhs=wkvT[:, kc, C + ch * 512 : C + (ch + 1) * 512],
                        start=(kc == 0),
                        stop=(kc == KC - 1),
                    )
                nc.scalar.copy(
                    out=vsbs[b][:L, ch * 512 : (ch + 1) * 512], in_=pt[:L, :]
                )

            # ---- attention pieces ----
            e01s = {}
            Rs = {}

            def scores_hp(u, hp):
                # PE: sT pair (row-tiled, M=128 w/ zero-padded k rows);
                # ACT: masked exp of full 128 rows -> bf16 e01
                b, j = units[u]
                mb, qT = mbs[b], qTs[u]
                s0 = ps.tile([128, 512], FP, tag="ps", name="s0")
                s1 = ps.tile([128, 512], FP, tag="ps", name="s1")
                nc.tensor.matmul(
                    s0[:],
                    lhsT=ktT[0:64, hp, b * 128 : (b + 1) * 128],
                    rhs=qT[0:64, hp, :],
                    start=True,
                    stop=True,
                )
                nc.tensor.matmul(
                    s1[:],
                    lhsT=ktT[64:128, hp, b * 128 : (b + 1) * 128],
                    rhs=qT[64:128, hp, :],
                    start=True,
                    stop=True,
                )
                e01 = sm.tile([128, 1024], BF, tag="e01", name="e01", bufs=5)
                nc.scalar.activation(
                    out=e01[:, 0:512], in_=s0[:], func=Exp, bias=mb[:, :],
                    scale=SCALE,
                )
                nc.scalar.activation(
                    out=e01[:, 512:1024], in_=s1[:], func=Exp, bias=mb[:, :],
                    scale=SCALE,
                )
                e01s[(u, hp)] = e01

            def rowsum_hp(u, hp):
                # GpSimd: softmax denominators, broadcast to all partitions
                e01 = e01s[(u, hp)]
                R = sm.tile([128, 1024], FP, tag="R", name="R", bufs=3)
                nc.gpsimd.partition_all_reduce(
                    R[:], e01[:], channels=128, reduce_op=bass_isa.ReduceOp.add
                )
                Rs[(u, hp)] = R

            def av_hp(u, hp, onormT):
                # PE: attn@v col-tiled pair; DVE/GpSimd: normalize
                b, j = units[u]
                vsb = vsbs[b]
                e01 = e01s.pop((u, hp))
                h0, h1 = 2 * hp, 2 * hp + 1
                ops_t = ps.tile([128, 512], FP, tag="ps", name="ops_t")
                nc.tensor.matmul(
                    ops_t[0:64, :],
                    lhsT=vsb[:L, h0 * DH : (h0 + 1) * DH],
                    rhs=e01[:L, 0:512],
                    start=True,
                    stop=True,
                )
                nc.tensor.matmul(
                    ops_t[64:128, :],
                    lhsT=vsb[:L, h1 * DH : (h1 + 1) * DH],
                    rhs=e01[:L, 512:1024],
                    start=True,
                    stop=True,
                )
                if USE_GPSIMD_SOFTMAX:
                    R = Rs.pop((u, hp))
                    nc.vector.reciprocal_approx_fast(out=R[:], in_=R[:])
                    nc.vector.tensor_mul(
                        out=onormT[0:64, hp, :],
                        in0=ops_t[0:64, :],
                        in1=R[0:64, 0:512],
                    )
                    nc.vector.tensor_mul(
                        out=onormT[64:128, hp, :],
                        in0=ops_t[64:128, :],
                        in1=R[64:128, 512:1024],
                    )
                else:
                    rps = ps.tile([128, 512], FP, tag="ps", name="rps")
                    nc.tensor.matmul(
                        rps[0:64, :], lhsT=ones_t[:L, :], rhs=e01[:L, 0:512],
                        start=True, stop=True,
                    )
                    nc.tensor.matmul(
                        rps[64:128, :], lhsT=ones_t[:L, :],
                        rhs=e01[:L, 512:1024], start=True, stop=True,
                    )
                    rr = sm.tile([128, 512], FP, tag="rr", name="rr", bufs=2)
                    nc.vector.reciprocal_approx_fast(out=rr[:], in_=rps[:])
                    nc.vector.tensor_mul(
                        out=onormT[:, hp, :], in0=ops_t[:], in1=rr[:]
                    )

            # out-projection: one (nsub, ch) group of 8 accumulating MMs
            proj_state = {}

            def proj_group(u, onormT, g):
                b, j = units[u]
                nsub, ch = divmod(g, 2)
                if ch == 0:
                    proj_state[u] = sm.tile([128, C], BF, tag="ysb", name="ysb", bufs=2)
                ysb = proj_state[u]
                pt = ps.tile([128, 512], FP, tag="ps", name="y_ps")
                for kc in range(KC):
                    nc.tensor.matmul(
                        pt[:],
                        lhsT=onormT[:, kc, nsub * 128 : (nsub + 1) * 128],
                        rhs=wpT[:, kc, ch * 512 : (ch + 1) * 512],
                        start=(kc == 0),
                        stop=(kc == KC - 1),
                    )
                nc.scalar.copy(out=ysb[:, ch * 512 : (ch + 1) * 512], in_=pt[:])
                if ch == 1:
                    row0 = j * NHALF + nsub * 128
                    nc.sync.dma_start(out=out_d[b, row0 : row0 + 128, :], in_=ysb[:])

            # ================= emission =================
            # prologue: x0 loads back-to-back on ACT ring; Wq blocks on SP
            # ring; casts chase loads on DVE; x0 transposes split across the
            # two rings so they run concurrently.
            for b in range(B_PER_CORE):
                mask_load(b)
            x_alloc(0)
            x_load(0)
            for kc in range(KC):
                w_load_chunk(wqT_d, wqT, kc)
            for s in range(4):
                x_cast(0, s)
                x_xpose(0, s, nc.sync if s % 2 == 0 else nc.scalar)
            # zero ktT pad columns (120:128 and 248:256 are never written by
            # the k-proj copies; scores matmuls read them as zero-k rows)
            nc.vector.memset(ktT[:, :, 120:128], 0.0)
            nc.vector.memset(ktT[:, :, 248:256], 0.0)

            # Q(0) then K-proj (cond + WkvK blocks stream in underneath)
            for m in range(KC):
                q_proj_chunk(0, m)
            for b in range(B_PER_CORE):
                cond_load(b)
            for kc in range(KC):
                w_load_chunk(wkvT_d, wkvT, kc, half=0)
            for m in range(KC):
                k_proj_chunk(m)
            for kc in range(KC):
                w_load_chunk(wkvT_d, wkvT, kc, half=1)
            for kc in range(KC):
                w_load_chunk(wpT_d, wpT, kc)

            # unit0 scores pass, first half (Wp + x1 stream in underneath;
            # x1 transposes go on the ACT ring so they can't head-block Wp)
            x_alloc(1)
            x_load(1)
            for hp in range(4):
                scores_hp(0, hp)
                x_cast(1, hp)
                x_xpose(1, hp, nc.scalar)

            # unit0 AV pass, interleaved with V-proj and Q(1). AV(0, hp<4)
            # only needs vsb ch0, so ch1/b1 V-projections are deferred past
            # them (their WkvV blocks arrive later).
            onormTs = {0: act.tile([128, KC, NHALF], BF, tag="onormT", name="onormT", bufs=3)}
            v_proj_chunk(0, 0)
            if DEBUG_DUMPS:
                nc.sync.dma_start(out=dbg["dbg_qT"], in_=qTs[0][:])
                nc.sync.dma_start(out=dbg["dbg_ktT"], in_=ktT[:])
                nc.sync.dma_start(out=dbg["dbg_xT"], in_=xTs[0][:])
                nc.sync.dma_start(out=dbg["dbg_e01"], in_=e01s[(0, 0)][:])
            for hp in range(4):
                if USE_GPSIMD_SOFTMAX:
                    rowsum_hp(0, hp)
                av_hp(0, hp, onormTs[0])
                q_proj_chunk(1, hp)
            for hp in range(4, HP):
                scores_hp(0, hp)
            v_proj_chunk(0, 1)
            if DEBUG_DUMPS:
                nc.sync.dma_start(out=dbg["dbg_vsb0"][:L, :], in_=vsbs[0][:L, :])
            v_proj_chunk(1, 0)
            v_proj_chunk(1, 1)
            for hp in range(4, HP):
                if USE_GPSIMD_SOFTMAX:
                    rowsum_hp(0, hp)
                av_hp(0, hp, onormTs[0])
                q_proj_chunk(1, hp)
            if DEBUG_DUMPS:
                nc.sync.dma_start(out=dbg["dbg_onormT"], in_=onormTs[0][:])
            xTs.pop(0)

            # steady units 1..3: proj is deferred by TWO units (proj(u-2)
            # interleaves unit u's attention) so Wp and ysb/store work move
            # out of the load-crunch window entirely.
            for u in range(1, len(units)):
                if u + 1 < len(units):
                    x_alloc(u + 1)
                onormTs[u] = act.tile(
                    [128, KC, NHALF], BF, tag="onormT", name="onormT", bufs=3
                )
                if u + 1 < len(units):
                    x_load(u + 1)
                pdone = u - 2
                pending = None
                for hp in range(HP):
                    scores_hp(u, hp)
                    if USE_GPSIMD_SOFTMAX:
                        rowsum_hp(u, hp)
                    if pdone >= 0:
                        proj_group(pdone, onormTs[pdone], hp)
                    if u + 1 < len(units) and hp < 4:
                        x_cast(u + 1, hp)
                        x_xpose(u + 1, hp, nc.sync)
                    if pending is not None:
                        av_hp(u, pending, onormTs[u])
                    pending = hp
                av_hp(u, pending, onormTs[u])
                qTs.pop(u - 1, None)
                if pdone >= 0:
                    onormTs.pop(pdone, None)
                xTs.pop(u, None)
                if u + 1 < len(units):
                    for m in range(KC):
                        q_proj_chunk(u + 1, m)

            # drain: projections of the last two units
            for u in (len(units) - 2, len(units) - 1):
                for g in range(8):
                    proj_group(u, onormTs[u], g)

    nc.compile()
    return nc


def get_nc():
    if "nc" not in _CACHE:
        _CACHE["nc"] = _build_nc()
    return _CACHE["nc"]


def make_in_maps(x, cond, mask, Wq, Wkv, Wp):
    x = np.ascontiguousarray(np.asarray(x, dtype=np.float32))
    cond = np.ascontiguousarray(np.asarray(cond, dtype=np.float32))
    mask = np.ascontiguousarray(np.asarray(mask, dtype=np.int32))
    WqT = np.ascontiguousarray(np.asarray(Wq, dtype=np.float32).T)
    WkvT = np.ascontiguousarray(np.asarray(Wkv, dtype=np.float32).T)
    WpT = np.ascontiguousarray(np.asarray(Wp, dtype=np.float32).T)
    in_maps = []
    for i in range(N_CORES):
        s = slice(i * B_PER_CORE, (i + 1) * B_PER_CORE)
        in_maps.append(
            {
                "x": x[s],
                "cond": cond[s],
                "mask": mask[s],
                "WqT": WqT,
                "WkvT": WkvT,
                "WpT": WpT,
            }
        )
    return in_maps


def run(x, cond, mask, Wq, Wkv, Wp, trace=False):
    from concourse import bass_utils

    nc = get_nc()
    in_maps = make_in_maps(x, cond, mask, Wq, Wkv, Wp)
    res = bass_utils.run_bass_kernel_spmd(
        nc, in_maps, core_ids=list(range(N_CORES)), trace=trace
    )
    out = np.concatenate(
        [np.asarray(res.results[i]["out"]) for i in range(N_CORES)], axis=0
    )
    return out.astype(np.float32), res


def kernel(x, cond, mask, Wq, bq, Wkv, bkv, Wp, bp):
    # bq/bkv/bp are zeros per the problem spec (fill: zeros) and are unused.
    out, _ = run(x, cond, mask, Wq, Wkv, Wp, trace=False)
    return out


# revision 33
# speedup vs baseline: 1.0291x; 1.0291x over previous
"""Trainium2 Bass kernel for MultiHeadCrossAttention.

Problem: y = proj(softmax(mask(q @ k^T / sqrt(Dh))) @ v) with
  x: (16, 1024, 1024) f32, cond: (16, 120, 1024) f32, mask: (16, 120) i32,
  Wq: (1024, 1024), Wkv: (2048, 1024), Wp: (1024, 1024); H=16 heads, Dh=64.
  Biases are all zeros per the problem spec and are skipped.

Sharding: pure data-parallel over batch B=16 -> 2 batches per core on 8
NeuronCores. No collectives; each core runs the same program (SPMD) on its
batch shard plus the full (replicated) weights.

Host-side prep (layout only): weights are staged to device DRAM already
transposed (W.T, f32) so the device loads W^T with natural, 512B-contiguous
strided DMA instead of on-chip XBAR transposition. Output is produced bf16
on device and upcast to f32 on host.

Per-core dataflow (everything "transposed" so each matmul contracts over the
partition dim):
  WqT/WkvT/WpT f32 block loads (SP ring) -> DVE cast to bf16 (resident)
  x f32 stage loads (ACT ring) -> DVE cast -> XBAR transpose (DVE ring)
  QT = WqT.T @ xT            [co, n]
  KT = WkvT(k).T @ condT     [co, l] (both batches merged, zero-padded to
                                      128-col blocks per batch)
  V  = condT.T @ WkvT(v)     [l, co]
  sT_h = KT_h.T @ QT_h       [l(128 rows, 8 zero-pad), n]
  e = Exp(sT/8 + maskbias)   (ACT; pad rows get -50 bias -> e ~ 0)
  R = partition_all_reduce(e01)  (GpSimd: softmax denominators broadcast to
                                  all 128 partitions -- off the PE)
  o~T_h = V_h.T @ e_h        [d, n] (head pairs via PE col-tiling)
  onormT = o~T * recip(R windows)  (DVE recip + DVE/GpSimd muls)
  y = onormT.T @ WpT         [n, co] -> bf16 SBUF -> DRAM.

Emission interleaves unit u's attention with unit u-1's out-projection and
unit u+1's Q-projection so the PE stream stays dense.
"""

import sys

for _p in ("/opt/trn_rl_repo", "/opt/pypackages"):
    if _p not in sys.path:
        sys.path.append(_p)

import numpy as np

B = 16
N_CORES = 8
B_PER_CORE = B // N_CORES  # 2
N = 1024
C = 1024
L = 120
H = 16
DH = C // H  # 64
SCALE = DH ** -0.5  # 0.125

KC = C // 128  # 8 c-chunks of 128
HP = H // 2  # 8 head pairs
NJ = 2  # n-halves per batch
NHALF = N // NJ  # 512
NEG = -50.0  # masked-logit bias; exp(s/8 - 50) ~ 0 vs reference's -inf

# Softmax row-sum strategy: True = GpSimd partition_all_reduce (PE saves the
# ones-broadcast matmuls), False = baseline-style PE ones-matmul broadcast.
USE_GPSIMD_SOFTMAX = False
DEBUG_DUMPS = False

_CACHE = {}


def _build_nc():
    import concourse.mybir as mybir
    import concourse.tile as tile
    from concourse import bacc, bass_isa

    FP = mybir.dt.float32
    BF = mybir.dt.bfloat16
    I32 = mybir.dt.int32
    Exp = mybir.ActivationFunctionType.Exp
    Alu = mybir.AluOpType

    nc = bacc.Bacc("TRN2", target_bir_lowering=False, debug=False)

    x_d = nc.dram_tensor("x", [B_PER_CORE, N, C], FP, kind="ExternalInput").ap()
    cond_d = nc.dram_tensor("cond", [B_PER_CORE, L, C], FP, kind="ExternalInput").ap()
    mask_d = nc.dram_tensor("mask", [B_PER_CORE, L], I32, kind="ExternalInput").ap()
    # Weights arrive HOST-TRANSPOSED: WqT = Wq.T [C, C], WkvT = Wkv.T
    # [C, 2C] (cols 0:C = Wk^T, C:2C = Wv^T), WpT = Wp.T [C, C].
    wqT_d = nc.dram_tensor("WqT", [C, C], FP, kind="ExternalInput").ap()
    wkvT_d = nc.dram_tensor("WkvT", [C, 2 * C], FP, kind="ExternalInput").ap()
    wpT_d = nc.dram_tensor("WpT", [C, C], FP, kind="ExternalInput").ap()
    out_d = nc.dram_tensor("out", [B_PER_CORE, N, C], BF, kind="ExternalOutput").ap()
    dbg = {}
    if DEBUG_DUMPS:
        for nm, shp, dt in [
            ("dbg_qT", [128, KC, NHALF], BF),
            ("dbg_ktT", [128, KC, 256], BF),
            ("dbg_vsb0", [128, C], BF),
            ("dbg_e01", [128, 1024], BF),
            ("dbg_R", [128, 1024], FP),
            ("dbg_onormT", [128, KC, NHALF], BF),
            ("dbg_xT", [128, KC, NHALF], BF),
        ]:
            dbg[nm] = nc.dram_tensor(nm, shp, dt, kind="ExternalOutput").ap()

    with tile.TileContext(nc) as tc:
        with (
            tc.tile_pool(name="wt", bufs=1) as wt,
            tc.tile_pool(name="wst", bufs=5) as wst,
            tc.tile_pool(name="xst", bufs=4) as xst,
            tc.tile_pool(name="act", bufs=2) as act,
            tc.tile_pool(name="sm", bufs=3) as sm,
            tc.tile_pool(name="ps", bufs=8, space="PSUM") as ps,
        ):
            # ---- resident bf16 transposed weights ----
            wqT = wt.tile([128, KC, C], BF, tag="wqT", name="wqT")
            wkvT = wt.tile([128, KC, 2 * C], BF, tag="wkvT", name="wkvT")
            wpT = wt.tile([128, KC, C], BF, tag="wpT", name="wpT")
            # per-core resident KV state (both batches)
            condT = wt.tile([128, KC, 256], BF, tag="condT", name="condT")
            ktT = wt.tile([128, KC, 256], BF, tag="ktT", name="ktT")
            vsbs = [
                wt.tile([128, C], BF, tag=f"vsb{b}", name=f"vsb{b}")
                for b in range(B_PER_CORE)
            ]
            mbs = [
                wt.tile([128, 1], FP, tag=f"mb{b}", name=f"mb{b}")
                for b in range(B_PER_CORE)
            ]
            if not USE_GPSIMD_SOFTMAX:
                ones_t = wt.tile([128, DH], BF, tag="ones_t", name="ones_t")
                nc.vector.memset(ones_t, 1.0)

            def w_load_chunk(dram2d, dstT, kc, half=None):
                # one kc row-chunk of a host-transposed weight: [128, 1024]
                # with 4KB-contiguous rows (full DMA descriptor rate), then
                # DVE cast into the resident bf16 tile. half selects the
                # K (0) / V (1) column half of WkvT.
                wf = wst.tile([128, C], FP, tag="wf", name="wf")
                src = dram2d[kc * 128 : (kc + 1) * 128, :]
                dst = dstT[:, kc, :]
                if half is not None:
                    src = src[:, half * C : (half + 1) * C]
                    dst = dstT[:, kc, half * C : (half + 1) * C]
                nc.sync.dma_start(out=wf[:], in_=src)
                if (kc + (0 if half is None else half)) % 2 == 0:
                    nc.vector.tensor_copy(out=dst, in_=wf[:])
                else:
                    nc.scalar.copy(out=dst, in_=wf[:])

            # ---- per-(batch, n-half) state ----
            units = [(b, j) for b in range(B_PER_CORE) for j in range(NJ)]
            xTs = {}
            qTs = {}

            def x_alloc(u):
                xTs[u] = act.tile([128, KC, NHALF], BF, tag="xT", name="xT")

            x_stage_bufs = {}

            def x_load(u):
                # whole n-half of x in ONE DMA (4KB rows): [128, 4, 1024]
                b, j = units[u]
                xf = xst.tile([128, 4, C], FP, tag="xfb", name="xfb", bufs=1)
                nc.scalar.dma_start(
                    out=xf[:],
                    in_=x_d[b, j * NHALF : (j + 1) * NHALF, :].rearrange(
                        "(s p) c -> p s c", p=128
                    ),
                )
                x_stage_bufs[u] = xf

            def x_cast(u, s):
                xf = x_stage_bufs[u]
                xb = xst.tile([128, C], BF, tag="xb", name="xb", bufs=2)
                nc.vector.tensor_copy(out=xb[:], in_=xf[:, s, :])
                x_stage_bufs[(u, s)] = xb

            def x_xpose(u, s, ring):
                xb = x_stage_bufs.pop((u, s))
                ring.dma_start_transpose(
                    xTs[u][:, :, s * 128 : (s + 1) * 128], xb[:]
                )

            def cond_load(b):
                # cond rows 0:120 (pad rows zeroed) -> bf16 -> XBAR into
                # condT cols [b*128, b*128+128)
                cf = xst.tile([128, C], FP, tag="cf", name="cond_f", bufs=1)
                nc.vector.memset(cf[:], 0.0)
                nc.sync.dma_start(out=cf[:L, :], in_=cond_d[b])
                cb = xst.tile([128, C], BF, tag="cb", name="cond_b", bufs=1)
                nc.vector.tensor_copy(out=cb[:], in_=cf[:])
                nc.scalar.dma_start_transpose(
                    condT[:, :, b * 128 : (b + 1) * 128], cb[:]
                )

            def mask_load(b):
                mi = wst.tile([128, 1], I32, tag="mi", name="mi")
                nc.sync.dma_start(out=mi[:L, :], in_=mask_d[b][:, None])
                mb = mbs[b]
                nc.vector.memset(mb[:], NEG)
                nc.vector.tensor_copy(out=mb[:L, :], in_=mi[:L, :])
                nc.vector.tensor_scalar(
                    mb[:L, :], mb[:L, :], -NEG, NEG, Alu.mult, Alu.add
                )

            def q_proj_chunk(u, m):
                # one 128-row chunk m of QT for unit u (8 accumulating MMs)
                if m == 0:
                    qTs[u] = act.tile([128, KC, NHALF], BF, tag="qT", name="qT", bufs=4)
                xT, qT = xTs[u], qTs[u]
                pt = ps.tile([128, 512], FP, tag="ps", name="q_ps")
                for kc in range(KC):
                    nc.tensor.matmul(
                        pt[:],
                        lhsT=wqT[:, kc, m * 128 : (m + 1) * 128],
                        rhs=xT[:, kc, :],
                        start=(kc == 0),
                        stop=(kc == KC - 1),
                    )
                nc.scalar.copy(out=qT[:, m, :], in_=pt[:])

            def k_proj_chunk(m):
                # both batches at once: rhs N=248 (b0 cols 0:120 + 8 zero
                # pads, b1 cols 128:248); ktT pads pre-zeroed.
                pt = ps.tile([128, 512], FP, tag="ps", name="kt_ps")
                for kc in range(KC):
                    nc.tensor.matmul(
                        pt[:, 0:248],
                        lhsT=wkvT[:, kc, m * 128 : (m + 1) * 128],
                        rhs=condT[:, kc, 0:248],
                        start=(kc == 0),
                        stop=(kc == KC - 1),
                    )
                nc.scalar.copy(
                    out=ktT[:, m, :].rearrange("p (s k) -> p s k", s=2)[
                        :, :, 0:120
                    ],
                    in_=pt[:, 0:256].rearrange("p (s k) -> p s k", s=2)[
                        :, :, 0:120
                    ],
                )

            def v_proj_chunk(b, ch):
                pt = ps.tile([128, 512], FP, tag="ps", name="v_ps")
                for kc in range(KC):
                    nc.tensor.matmul(
                        pt[:L, :],
                        lhsT=condT[:, kc, b * 128 : b * 128 + L],
                        rhs=wkvT[:, kc, C + ch * 512 : C + (ch + 1) * 512],
                        start=(kc == 0),
                        stop=(kc == KC - 1),
                    )
                nc.scalar.copy(
                    out=vsbs[b][:L, ch * 512 : (ch + 1) * 512], in_=pt[:L, :]
                )

            # ---- attention pieces ----
            e01s = {}
            Rs = {}

            def scores_hp(u, hp):
                # PE: sT pair (row-tiled, M=128 w/ zero-padded k rows);
                # ACT: masked exp of full 128 rows -> bf16 e01
                b, j = units[u]
                mb, qT = mbs[b], qTs[u]
                s0 = ps.tile([128, 512], FP, tag="ps", name="s0")
                s1 = ps.tile([128, 512], FP, tag="ps", name="s1")
                nc.tensor.matmul(
                    s0[:],
                    lhsT=ktT[0:64, hp, b * 128 : (b + 1) * 128],
                    rhs=qT[0:64, hp, :],
                    start=True,
                    stop=True,
                )
                nc.tensor.matmul(
                    s1[:],
                    lhsT=ktT[64:128, hp, b * 128 : (b + 1) * 128],
                    rhs=qT[64:128, hp, :],
                    start=True,
                    stop=True,
                )
                e01 = sm.tile([128, 1024], BF, tag="e01", name="e01", bufs=5)
                nc.scalar.activation(
                    out=e01[:, 0:512], in_=s0[:], func=Exp, bias=mb[:, :],
                    scale=SCALE,
                )
                nc.scalar.activation(
                    out=e01[:, 512:1024], in_=s1[:], func=Exp, bias=mb[:, :],
                    scale=SCALE,
                )
                e01s[(u, hp)] = e01

            def rowsum_hp(u, hp):
                # GpSimd: softmax denominators, broadcast to all partitions
                e01 = e01s[(u, hp)]
                R = sm.tile([128, 1024], FP, tag="R", name="R", bufs=3)
                nc.gpsimd.partition_all_reduce(
                    R[:], e01[:], channels=128, reduce_op=bass_isa.ReduceOp.add
                )
                Rs[(u, hp)] = R

            def av_hp(u, hp, onormT):
                # PE: attn@v col-tiled pair; DVE/GpSimd: normalize
                b, j = units[u]
                vsb = vsbs[b]
                e01 = e01s.pop((u, hp))
                h0, h1 = 2 * hp, 2 * hp + 1
                ops_t = ps.tile([128, 512], FP, tag="ps", name="ops_t")
                nc.tensor.matmul(
                    ops_t[0:64, :],
                    lhsT=vsb[:L, h0 * DH : (h0 + 1) * DH],
                    rhs=e01[:L, 0:512],
                    start=True,
                    stop=True,
                )
                nc.tensor.matmul(
                    ops_t[64:128, :],
                    lhsT=vsb[:L, h1 * DH : (h1 + 1) * DH],
                    rhs=e01[:L, 512:1024],
                    start=True,
                    stop=True,
                )
                if USE_GPSIMD_SOFTMAX:
                    R = Rs.pop((u, hp))
                    nc.vector.reciprocal_approx_fast(out=R[:], in_=R[:])
                    nc.vector.tensor_mul(
                        out=onormT[0:64, hp, :],
                        in0=ops_t[0:64, :],
                        in1=R[0:64, 0:512],
                    )
                    nc.vector.tensor_mul(
                        out=onormT[64:128, hp, :],
                        in0=ops_t[64:128, :],
                        in1=R[64:128, 512:1024],
                    )
                else:
                    rps = ps.tile([128, 512], FP, tag="ps", name="rps")
                    nc.tensor.matmul(
                        rps[0:64, :], lhsT=ones_t[:L, :], rhs=e01[:L, 0:512],
                        start=True, stop=True,
                    )
                    nc.tensor.matmul(
                        rps[64:128, :], lhsT=ones_t[:L, :],
                        rhs=e01[:L, 512:1024], start=True, stop=True,
                    )
                    rr = sm.tile([128, 512], FP, tag="rr", name="rr", bufs=2)
                    nc.vector.reciprocal_approx_fast(out=rr[:], in_=rps[:])
                    nc.vector.tensor_mul(
                        out=onormT[:, hp, :], in0=ops_t[:], in1=rr[:]
                    )

            # out-projection: one (nsub, ch) group of 8 accumulating MMs
            proj_state = {}

            def proj_group(u, onormT, g):
                b, j = units[u]
                nsub, ch = divmod(g, 2)
                if ch == 0:
                    proj_state[u] = sm.tile([128, C], BF, tag="ysb", name="ysb", bufs=2)
                ysb = proj_state[u]
                pt = ps.tile([128, 512], FP, tag="ps", name="y_ps")
                for kc in range(KC):
                    nc.tensor.matmul(
                        pt[:],
                        lhsT=onormT[:, kc, nsub * 128 : (nsub + 1) * 128],
                        rhs=wpT[:, kc, ch * 512 : (ch + 1) * 512],
                        start=(kc == 0),
                        stop=(kc == KC - 1),
                    )
                nc.scalar.copy(out=ysb[:, ch * 512 : (ch + 1) * 512], in_=pt[:])
                if ch == 1:
                    row0 = j * NHALF + nsub * 128
                    nc.sync.dma_start(out=out_d[b, row0 : row0 + 128, :], in_=ysb[:])

            # ================= emission =================
            # Phase 1 — Q-projection marathon: qproj(0..3) back-to-back on
            # the PE. Only Wq (first 4MB) and the x stream are needed, so
            # cond/Wkv/Wp can trickle in underneath with big margins.
            for b in range(B_PER_CORE):
                mask_load(b)
            x_alloc(0)
            x_load(0)
            for kc in range(KC):
                w_load_chunk(wqT_d, wqT, kc)
            for s in range(4):
                x_cast(0, s)
                x_xpose(0, s, nc.sync if s % 2 == 0 else nc.scalar)
            # zero ktT pad columns (120:128 and 248:256 are never written by
            # the k-proj copies; scores matmuls read them as zero-k rows)
            nc.vector.memset(ktT[:, :, 120:128], 0.0)
            nc.vector.memset(ktT[:, :, 248:256], 0.0)

            x_alloc(1)
            x_load(1)
            for m in range(KC):
                q_proj_chunk(0, m)
            for b in range(B_PER_CORE):
                cond_load(b)
            for s in range(4):
                x_cast(1, s)
                x_xpose(1, s, nc.sync)
            for kc in range(KC):
                w_load_chunk(wkvT_d, wkvT, kc, half=0)

            x_alloc(2)
            x_load(2)
            for m in range(KC):
                q_proj_chunk(1, m)
            xTs.pop(0)
            for s in range(4):
                x_cast(2, s)
                x_xpose(2, s, nc.sync)

            x_alloc(3)
            x_load(3)
            for m in range(KC):
                q_proj_chunk(2, m)
            xTs.pop(1)
            for s in range(4):
                x_cast(3, s)
                x_xpose(3, s, nc.sync)
            for kc in range(KC):
                w_load_chunk(wkvT_d, wkvT, kc, half=1)

            for m in range(KC):
                q_proj_chunk(3, m)
            xTs.pop(2)
            for kc in range(KC):
                w_load_chunk(wpT_d, wpT, kc)
            xTs.pop(3)

            # Phase 2 — KV projections (both batches)
            for m in range(KC):
                k_proj_chunk(m)
            v_proj_chunk(0, 0)
            v_proj_chunk(0, 1)
            v_proj_chunk(1, 0)
            v_proj_chunk(1, 1)

            # Phase 3 — attention units; proj(u-1) interleaves unit u
            onormTs = {}
            for u in range(len(units)):
                onormTs[u] = act.tile(
                    [128, KC, NHALF], BF, tag="onormT", name="onormT", bufs=2
                )
                pending = None
                for hp in range(HP):
                    scores_hp(u, hp)
                    if USE_GPSIMD_SOFTMAX:
                        rowsum_hp(u, hp)
                    if u >= 1:
                        proj_group(u - 1, onormTs[u - 1], hp)
                    if pending is not None:
                        av_hp(u, pending, onormTs[u])
                    pending = hp
                av_hp(u, pending, onormTs[u])
                qTs.pop(u, None)
                if u >= 1:
                    onormTs.pop(u - 1)

            # drain: projection of the last unit
            u = len(units) - 1
            for g in range(8):
                proj_group(u, onormTs[u], g)

    nc.compile()
    return nc


def get_nc():
    if "nc" not in _CACHE:
        _CACHE["nc"] = _build_nc()
    return _CACHE["nc"]


def make_in_maps(x, cond, mask, Wq, Wkv, Wp):
    x = np.ascontiguousarray(np.asarray(x, dtype=np.float32))
    cond = np.ascontiguousarray(np.asarray(cond, dtype=np.float32))
    mask = np.ascontiguousarray(np.asarray(mask, dtype=np.int32))
    WqT = np.ascontiguousarray(np.asarray(Wq, dtype=np.float32).T)
    WkvT = np.ascontiguousarray(np.asarray(Wkv, dtype=np.float32).T)
    WpT = np.ascontiguousarray(np.asarray(Wp, dtype=np.float32).T)
    in_maps = []
    for i in range(N_CORES):
        s = slice(i * B_PER_CORE, (i + 1) * B_PER_CORE)
        in_maps.append(
            {
                "x": x[s],
                "cond": cond[s],
                "mask": mask[s],
                "WqT": WqT,
                "WkvT": WkvT,
                "WpT": WpT,
            }
        )
    return in_maps


def run(x, cond, mask, Wq, Wkv, Wp, trace=False):
    from concourse import bass_utils

    nc = get_nc()
    in_maps = make_in_maps(x, cond, mask, Wq, Wkv, Wp)
    res = bass_utils.run_bass_kernel_spmd(
        nc, in_maps, core_ids=list(range(N_CORES)), trace=trace
    )
    out = np.concatenate(
        [np.asarray(res.results[i]["out"]) for i in range(N_CORES)], axis=0
    )
    return out.astype(np.float32), res


def kernel(x, cond, mask, Wq, bq, Wkv, bkv, Wp, bp):
    # bq/bkv/bp are zeros per the problem spec (fill: zeros) and are unused.
    out, _ = run(x, cond, mask, Wq, Wkv, Wp, trace=False)
    return out


# revision 35
# speedup vs baseline: 1.0292x; 1.0000x over previous
"""Trainium2 Bass kernel for MultiHeadCrossAttention.

Problem: y = proj(softmax(mask(q @ k^T / sqrt(Dh))) @ v) with
  x: (16, 1024, 1024) f32, cond: (16, 120, 1024) f32, mask: (16, 120) i32,
  Wq: (1024, 1024), Wkv: (2048, 1024), Wp: (1024, 1024); H=16 heads, Dh=64.
  Biases are all zeros per the problem spec and are skipped.

Sharding: pure data-parallel over batch B=16 -> 2 batches per core on 8
NeuronCores. No collectives; each core runs the same program (SPMD) on its
batch shard plus the full (replicated) weights.

Host-side prep (layout only): weights are staged to device DRAM already
transposed (W.T, f32) so the device loads W^T with natural, 512B-contiguous
strided DMA instead of on-chip XBAR transposition. Output is produced bf16
on device and upcast to f32 on host.

Per-core dataflow (everything "transposed" so each matmul contracts over the
partition dim):
  WqT/WkvT/WpT f32 block loads (SP ring) -> DVE cast to bf16 (resident)
  x f32 stage loads (ACT ring) -> DVE cast -> XBAR transpose (DVE ring)
  QT = WqT.T @ xT            [co, n]
  KT = WkvT(k).T @ condT     [co, l] (both batches merged, zero-padded to
                                      128-col blocks per batch)
  V  = condT.T @ WkvT(v)     [l, co]
  sT_h = KT_h.T @ QT_h       [l(128 rows, 8 zero-pad), n]
  e = Exp(sT/8 + maskbias)   (ACT; pad rows get -50 bias -> e ~ 0)
  R = partition_all_reduce(e01)  (GpSimd: softmax denominators broadcast to
                                  all 128 partitions -- off the PE)
  o~T_h = V_h.T @ e_h        [d, n] (head pairs via PE col-tiling)
  onormT = o~T * recip(R windows)  (DVE recip + DVE/GpSimd muls)
  y = onormT.T @ WpT         [n, co] -> bf16 SBUF -> DRAM.

Emission interleaves unit u's attention with unit u-1's out-projection and
unit u+1's Q-projection so the PE stream stays dense.
"""

import sys

for _p in ("/opt/trn_rl_repo", "/opt/pypackages"):
    if _p not in sys.path:
        sys.path.append(_p)

import numpy as np

B = 16
N_CORES = 8
B_PER_CORE = B // N_CORES  # 2
N = 1024
C = 1024
L = 120
H = 16
DH = C // H  # 64
SCALE = DH ** -0.5  # 0.125

KC = C // 128  # 8 c-chunks of 128
HP = H // 2  # 8 head pairs
NJ = 2  # n-halves per batch
NHALF = N // NJ  # 512
NEG = -50.0  # masked-logit bias; exp(s/8 - 50) ~ 0 vs reference's -inf

# Softmax row-sum strategy: True = GpSimd partition_all_reduce (PE saves the
# ones-broadcast matmuls), False = baseline-style PE ones-matmul broadcast.
USE_GPSIMD_SOFTMAX = False
DEBUG_DUMPS = False

_CACHE = {}


def _build_nc():
    import concourse.mybir as mybir
    import concourse.tile as tile
    from concourse import bacc, bass_isa

    FP = mybir.dt.float32
    BF = mybir.dt.bfloat16
    I32 = mybir.dt.int32
    Exp = mybir.ActivationFunctionType.Exp
    Alu = mybir.AluOpType

    nc = bacc.Bacc("TRN2", target_bir_lowering=False, debug=False)

    x_d = nc.dram_tensor("x", [B_PER_CORE, N, C], FP, kind="ExternalInput").ap()
    cond_d = nc.dram_tensor("cond", [B_PER_CORE, L, C], FP, kind="ExternalInput").ap()
    mask_d = nc.dram_tensor("mask", [B_PER_CORE, L], I32, kind="ExternalInput").ap()
    # Weights arrive HOST-TRANSPOSED: WqT = Wq.T [C, C], WkvT = Wkv.T
    # [C, 2C] (cols 0:C = Wk^T, C:2C = Wv^T), WpT = Wp.T [C, C].
    wqT_d = nc.dram_tensor("WqT", [C, C], FP, kind="ExternalInput").ap()
    wkvT_d = nc.dram_tensor("WkvT", [C, 2 * C], FP, kind="ExternalInput").ap()
    wpT_d = nc.dram_tensor("WpT", [C, C], FP, kind="ExternalInput").ap()
    out_d = nc.dram_tensor("out", [B_PER_CORE, N, C], BF, kind="ExternalOutput").ap()
    dbg = {}
    if DEBUG_DUMPS:
        for nm, shp, dt in [
            ("dbg_qT", [128, KC, NHALF], BF),
            ("dbg_ktT", [128, KC, 256], BF),
            ("dbg_vsb0", [128, C], BF),
            ("dbg_e01", [128, 1024], BF),
            ("dbg_R", [128, 1024], FP),
            ("dbg_onormT", [128, KC, NHALF], BF),
            ("dbg_xT", [128, KC, NHALF], BF),
        ]:
            dbg[nm] = nc.dram_tensor(nm, shp, dt, kind="ExternalOutput").ap()

    with tile.TileContext(nc) as tc:
        with (
            tc.tile_pool(name="wt", bufs=1) as wt,
            tc.tile_pool(name="wst", bufs=5) as wst,
            tc.tile_pool(name="xst", bufs=4) as xst,
            tc.tile_pool(name="act", bufs=2) as act,
            tc.tile_pool(name="sm", bufs=3) as sm,
            tc.tile_pool(name="ps", bufs=8, space="PSUM") as ps,
        ):
            # ---- resident bf16 transposed weights ----
            wqT = wt.tile([128, KC, C], BF, tag="wqT", name="wqT")
            wkvT = wt.tile([128, KC, 2 * C], BF, tag="wkvT", name="wkvT")
            wpT = wt.tile([128, KC, C], BF, tag="wpT", name="wpT")
            # per-core resident KV state (both batches)
            condT = wt.tile([128, KC, 256], BF, tag="condT", name="condT")
            ktT = wt.tile([128, KC, 256], BF, tag="ktT", name="ktT")
            vsbs = [
                wt.tile([128, C], BF, tag=f"vsb{b}", name=f"vsb{b}")
                for b in range(B_PER_CORE)
            ]
            mbs = [
                wt.tile([128, 1], FP, tag=f"mb{b}", name=f"mb{b}")
                for b in range(B_PER_CORE)
            ]
            if not USE_GPSIMD_SOFTMAX:
                ones_t = wt.tile([128, DH], BF, tag="ones_t", name="ones_t")
                nc.vector.memset(ones_t, 1.0)

            def w_load_chunk(dram2d, dstT, kc, half=None):
                # one kc row-chunk of a host-transposed weight: [128, 1024]
                # with 4KB-contiguous rows (full DMA descriptor rate), then
                # DVE cast into the resident bf16 tile. half selects the
                # K (0) / V (1) column half of WkvT.
                wf = wst.tile([128, C], FP, tag="wf", name="wf")
                src = dram2d[kc * 128 : (kc + 1) * 128, :]
                dst = dstT[:, kc, :]
                if half is not None:
                    src = src[:, half * C : (half + 1) * C]
                    dst = dstT[:, kc, half * C : (half + 1) * C]
                nc.sync.dma_start(out=wf[:], in_=src)
                if (kc + (0 if half is None else half)) % 2 == 0:
                    nc.vector.tensor_copy(out=dst, in_=wf[:])
                else:
                    nc.scalar.copy(out=dst, in_=wf[:])

            # ---- per-(batch, n-half) state ----
            units = [(b, j) for b in range(B_PER_CORE) for j in range(NJ)]
            xTs = {}
            qTs = {}

            def x_alloc(u):
                xTs[u] = act.tile([128, KC, NHALF], BF, tag="xT", name="xT")

            x_stage_bufs = {}

            def x_load(u):
                # whole n-half of x in ONE DMA (4KB rows): [128, 4, 1024]
                b, j = units[u]
                xf = xst.tile([128, 4, C], FP, tag="xfb", name="xfb", bufs=1)
                nc.scalar.dma_start(
                    out=xf[:],
                    in_=x_d[b, j * NHALF : (j + 1) * NHALF, :].rearrange(
                        "(s p) c -> p s c", p=128
                    ),
                )
                x_stage_bufs[u] = xf

            def x_cast(u, s):
                xf = x_stage_bufs[u]
                xb = xst.tile([128, C], BF, tag="xb", name="xb", bufs=2)
                nc.vector.tensor_copy(out=xb[:], in_=xf[:, s, :])
                x_stage_bufs[(u, s)] = xb

            def x_xpose(u, s, ring):
                xb = x_stage_bufs.pop((u, s))
                ring.dma_start_transpose(
                    xTs[u][:, :, s * 128 : (s + 1) * 128], xb[:]
                )

            def cond_load(b):
                # cond rows 0:120 (pad rows zeroed) -> bf16 -> XBAR into
                # condT cols [b*128, b*128+128)
                cf = xst.tile([128, C], FP, tag="cf", name="cond_f", bufs=1)
                nc.vector.memset(cf[:], 0.0)
                nc.sync.dma_start(out=cf[:L, :], in_=cond_d[b])
                cb = xst.tile([128, C], BF, tag="cb", name="cond_b", bufs=1)
                nc.vector.tensor_copy(out=cb[:], in_=cf[:])
                nc.scalar.dma_start_transpose(
                    condT[:, :, b * 128 : (b + 1) * 128], cb[:]
                )

            def mask_load(b):
                mi = wst.tile([128, 1], I32, tag="mi", name="mi")
                nc.sync.dma_start(out=mi[:L, :], in_=mask_d[b][:, None])
                mb = mbs[b]
                nc.vector.memset(mb[:], NEG)
                nc.vector.tensor_copy(out=mb[:L, :], in_=mi[:L, :])
                nc.vector.tensor_scalar(
                    mb[:L, :], mb[:L, :], -NEG, NEG, Alu.mult, Alu.add
                )

            def q_proj_chunk(u, m):
                # one 128-row chunk m of QT for unit u (8 accumulating MMs)
                if m == 0:
                    qTs[u] = act.tile([128, KC, NHALF], BF, tag="qT", name="qT", bufs=3)
                xT, qT = xTs[u], qTs[u]
                pt = ps.tile([128, 512], FP, tag="ps", name="q_ps")
                for kc in range(KC):
                    nc.tensor.matmul(
                        pt[:],
                        lhsT=wqT[:, kc, m * 128 : (m + 1) * 128],
                        rhs=xT[:, kc, :],
                        start=(kc == 0),
                        stop=(kc == KC - 1),
                    )
                nc.scalar.copy(out=qT[:, m, :], in_=pt[:])

            def k_proj_chunk(m):
                # both batches at once: rhs N=248 (b0 cols 0:120 + 8 zero
                # pads, b1 cols 128:248); ktT pads pre-zeroed.
                pt = ps.tile([128, 512], FP, tag="ps", name="kt_ps")
                for kc in range(KC):
                    nc.tensor.matmul(
                        pt[:, 0:248],
                        lhsT=wkvT[:, kc, m * 128 : (m + 1) * 128],
                        rhs=condT[:, kc, 0:248],
                        start=(kc == 0),
                        stop=(kc == KC - 1),
                    )
                nc.scalar.copy(
                    out=ktT[:, m, :].rearrange("p (s k) -> p s k", s=2)[
                        :, :, 0:120
                    ],
                    in_=pt[:, 0:256].rearrange("p (s k) -> p s k", s=2)[
                        :, :, 0:120
                    ],
                )

            def v_proj_chunk(b, ch):
                pt = ps.tile([128, 512], FP, tag="ps", name="v_ps")
                for kc in range(KC):
                    nc.tensor.matmul(
                        pt[:L, :],
                        lhsT=condT[:, kc, b * 128 : b * 128 + L],
                        rhs=wkvT[:, kc, C + ch * 512 : C + (ch + 1) * 512],
                        start=(kc == 0),
                        stop=(kc == KC - 1),
                    )
                nc.scalar.copy(
                    out=vsbs[b][:L, ch * 512 : (ch + 1) * 512], in_=pt[:L, :]
                )

            # ---- attention pieces ----
            e01s = {}
            Rs = {}

            def scores_hp(u, hp):
                # PE: sT pair (row-tiled, M=128 w/ zero-padded k rows);
                # ACT: masked exp of full 128 rows -> bf16 e01
                b, j = units[u]
                mb, qT = mbs[b], qTs[u]
                s0 = ps.tile([128, 512], FP, tag="ps", name="s0")
                s1 = ps.tile([128, 512], FP, tag="ps", name="s1")
                nc.tensor.matmul(
                    s0[:],
                    lhsT=ktT[0:64, hp, b * 128 : (b + 1) * 128],
                    rhs=qT[0:64, hp, :],
                    start=True,
                    stop=True,
                )
                nc.tensor.matmul(
                    s1[:],
                    lhsT=ktT[64:128, hp, b * 128 : (b + 1) * 128],
                    rhs=qT[64:128, hp, :],
                    start=True,
                    stop=True,
                )
                e01 = sm.tile([128, 1024], BF, tag="e01", name="e01", bufs=6)
                nc.scalar.activation(
                    out=e01[:, 0:512], in_=s0[:], func=Exp, bias=mb[:, :],
                    scale=SCALE,
                )
                nc.scalar.activation(
                    out=e01[:, 512:1024], in_=s1[:], func=Exp, bias=mb[:, :],
                    scale=SCALE,
                )
                e01s[(u, hp)] = e01

            def rowsum_hp(u, hp):
                # GpSimd: softmax denominators, broadcast to all partitions
                e01 = e01s[(u, hp)]
                R = sm.tile([128, 1024], FP, tag="R", name="R", bufs=3)
                nc.gpsimd.partition_all_reduce(
                    R[:], e01[:], channels=128, reduce_op=bass_isa.ReduceOp.add
                )
                Rs[(u, hp)] = R

            def av_hp(u, hp, onormT):
                # PE: attn@v col-tiled pair; DVE/GpSimd: normalize
                b, j = units[u]
                vsb = vsbs[b]
                e01 = e01s.pop((u, hp))
                h0, h1 = 2 * hp, 2 * hp + 1
                ops_t = ps.tile([128, 512], FP, tag="ps", name="ops_t")
                nc.tensor.matmul(
                    ops_t[0:64, :],
                    lhsT=vsb[:L, h0 * DH : (h0 + 1) * DH],
                    rhs=e01[:L, 0:512],
                    start=True,
                    stop=True,
                )
                nc.tensor.matmul(
                    ops_t[64:128, :],
                    lhsT=vsb[:L, h1 * DH : (h1 + 1) * DH],
                    rhs=e01[:L, 512:1024],
                    start=True,
                    stop=True,
                )
                if USE_GPSIMD_SOFTMAX:
                    R = Rs.pop((u, hp))
                    nc.vector.reciprocal_approx_fast(out=R[:], in_=R[:])
                    nc.vector.tensor_mul(
                        out=onormT[0:64, hp, :],
                        in0=ops_t[0:64, :],
                        in1=R[0:64, 0:512],
                    )
                    nc.vector.tensor_mul(
                        out=onormT[64:128, hp, :],
                        in0=ops_t[64:128, :],
                        in1=R[64:128, 512:1024],
                    )
                else:
                    rps = ps.tile([128, 512], FP, tag="ps", name="rps")
                    nc.tensor.matmul(
                        rps[0:64, :], lhsT=ones_t[:L, :], rhs=e01[:L, 0:512],
                        start=True, stop=True,
                    )
                    nc.tensor.matmul(
                        rps[64:128, :], lhsT=ones_t[:L, :],
                        rhs=e01[:L, 512:1024], start=True, stop=True,
                    )
                    rr = sm.tile([128, 512], FP, tag="rr", name="rr", bufs=2)
                    nc.vector.reciprocal_approx_fast(out=rr[:], in_=rps[:])
                    nc.vector.tensor_mul(
                        out=onormT[:, hp, :], in0=ops_t[:], in1=rr[:]
                    )

            # out-projection: one (nsub, ch) group of 8 accumulating MMs
            proj_state = {}

            def proj_group(u, onormT, g):
                b, j = units[u]
                nsub, ch = divmod(g, 2)
                if ch == 0:
                    proj_state[u] = sm.tile([128, C], BF, tag="ysb", name="ysb", bufs=2)
                ysb = proj_state[u]
                pt = ps.tile([128, 512], FP, tag="ps", name="y_ps")
                for kc in range(KC):
                    nc.tensor.matmul(
                        pt[:],
                        lhsT=onormT[:, kc, nsub * 128 : (nsub + 1) * 128],
                        rhs=wpT[:, kc, ch * 512 : (ch + 1) * 512],
                        start=(kc == 0),
                        stop=(kc == KC - 1),
                    )
                nc.scalar.copy(out=ysb[:, ch * 512 : (ch + 1) * 512], in_=pt[:])
                if ch == 1:
                    row0 = j * NHALF + nsub * 128
                    nc.sync.dma_start(out=out_d[b, row0 : row0 + 128, :], in_=ysb[:])

            # ================= emission =================
            # Schedule: qproj(0), qproj(1) fill the PE while Wq+x arrive
            # (first in the stream); kproj/vproj run as Wkv lands; the four
            # attention units follow with proj(u-1) interleaved one unit
            # behind and qproj(u+1) woven into the AV passes.
            for b in range(B_PER_CORE):
                mask_load(b)
            x_alloc(0)
            x_load(0)
            for kc in range(KC):
                w_load_chunk(wqT_d, wqT, kc)
            for s in range(4):
                x_cast(0, s)
                x_xpose(0, s, nc.sync)
            # zero ktT pad columns (120:128 and 248:256 are never written by
            # the k-proj copies; scores matmuls read them as zero-k rows)
            nc.vector.memset(ktT[:, :, 120:128], 0.0)
            nc.vector.memset(ktT[:, :, 248:256], 0.0)

            x_alloc(1)
            x_load(1)
            for m in range(KC):
                q_proj_chunk(0, m)
            for b in range(B_PER_CORE):
                cond_load(b)
            for s in range(4):
                x_cast(1, s)
                x_xpose(1, s, nc.sync)
            for kc in range(KC):
                w_load_chunk(wkvT_d, wkvT, kc, half=0)

            x_alloc(2)
            x_load(2)
            for m in range(KC):
                q_proj_chunk(1, m)
            xTs.pop(0)
            for s in range(4):
                x_cast(2, s)
                x_xpose(2, s, nc.sync)
            for kc in range(KC):
                w_load_chunk(wkvT_d, wkvT, kc, half=1)

            for m in range(KC):
                k_proj_chunk(m)
            xTs.pop(1)

            # unit0 scores pass; Wp streams underneath
            onormTs = {0: act.tile([128, KC, NHALF], BF, tag="onormT", name="onormT", bufs=2)}
            for hp in range(HP):
                scores_hp(0, hp)
                if hp < 4:
                    w_load_chunk(wpT_d, wpT, hp)
            v_proj_chunk(0, 0)
            v_proj_chunk(0, 1)
            v_proj_chunk(1, 0)
            v_proj_chunk(1, 1)
            for kc in range(4, KC):
                w_load_chunk(wpT_d, wpT, kc)

            # unit0 AV pass + qproj(2); x3 streams underneath
            x_alloc(3)
            x_load(3)
            for hp in range(HP):
                if USE_GPSIMD_SOFTMAX:
                    rowsum_hp(0, hp)
                av_hp(0, hp, onormTs[0])
                q_proj_chunk(2, hp)
                if hp < 4:
                    x_cast(3, hp)
                    x_xpose(3, hp, nc.sync)
            xTs.pop(2)
            qTs.pop(0, None)

            # units 1..3: scores + proj(u-1) + AV; qproj(3) after unit1
            for u in range(1, len(units)):
                onormTs[u] = act.tile(
                    [128, KC, NHALF], BF, tag="onormT", name="onormT", bufs=2
                )
                pending = None
                for hp in range(HP):
                    scores_hp(u, hp)
                    if USE_GPSIMD_SOFTMAX:
                        rowsum_hp(u, hp)
                    proj_group(u - 1, onormTs[u - 1], hp)
                    if pending is not None:
                        av_hp(u, pending, onormTs[u])
                    pending = hp
                av_hp(u, pending, onormTs[u])
                qTs.pop(u, None)
                onormTs.pop(u - 1)
                if u == 1:
                    for m in range(KC):
                        q_proj_chunk(3, m)
                    xTs.pop(3)

            # drain: projection of the last unit
            u = len(units) - 1
            for g in range(8):
                proj_group(u, onormTs[u], g)

    nc.compile()
    return nc


def get_nc():
    if "nc" not in _CACHE:
        _CACHE["nc"] = _build_nc()
    return _CACHE["nc"]


def make_in_maps(x, cond, mask, Wq, Wkv, Wp):
    x = np.ascontiguousarray(np.asarray(x, dtype=np.float32))
    cond = np.ascontiguousarray(np.asarray(cond, dtype=np.float32))
    mask = np.ascontiguousarray(np.asarray(mask, dtype=np.int32))
    WqT = np.ascontiguousarray(np.asarray(Wq, dtype=np.float32).T)
    WkvT = np.ascontiguousarray(np.asarray(Wkv, dtype=np.float32).T)
    WpT = np.ascontiguousarray(np.asarray(Wp, dtype=np.float32).T)
    in_maps = []
    for i in range(N_CORES):
        s = slice(i * B_PER_CORE, (i + 1) * B_PER_CORE)
        in_maps.append(
            {
                "x": x[s],
                "cond": cond[s],
                "mask": mask[s],
                "WqT": WqT,
                "WkvT": WkvT,
                "WpT": WpT,
            }
        )
    return in_maps


def run(x, cond, mask, Wq, Wkv, Wp, trace=False):
    from concourse import bass_utils

    nc = get_nc()
    in_maps = make_in_maps(x, cond, mask, Wq, Wkv, Wp)
    res = bass_utils.run_bass_kernel_spmd(
        nc, in_maps, core_ids=list(range(N_CORES)), trace=trace
    )
    out = np.concatenate(
        [np.asarray(res.results[i]["out"]) for i in range(N_CORES)], axis=0
    )
    return out.astype(np.float32), res


def kernel(x, cond, mask, Wq, bq, Wkv, bkv, Wp, bp):
    # bq/bkv/bp are zeros per the problem spec (fill: zeros) and are unused.
    out, _ = run(x, cond, mask, Wq, Wkv, Wp, trace=False)
    return out
